# revision 10
# baseline (speedup 1.0000x reference)
"""Multi-head causal attention (B=2, S=2048, D=1024, H=16, Dh=64) on 8 TRN2
NeuronCores.

Sharding: core c = 4*b + g handles batch b (2-way data parallel) and head
group g (4 heads = a 256-column slice of W_q/W_k/W_v, and the matching
256-row slice of W_o).  Each core returns a partial output [S, D]; the host
sums the 4 partials per batch and adds b_o.

On-core pipeline (per core):
  1. QKV projections as 3-term fp8 hi+lo DoubleRow matmuls:
     x@W ~= x8@W8 + x8@Wr + xr@W8 (residuals quantized unscaled), psum f32,
     descale during the PSUM->SBUF copy to bf16 (q also folds 1/sqrt(Dh)).
     Q^T/K^T land [dq, seq] (dims on partitions), V lands [seq, dv].
  2. Scores computed transposed per (head, 1024-query chunk, 128-key tile):
     S^T[k,q] = K^T.T Q^T in bf16.  The causal mask is folded in as one
     extra 128-wide matmul: with A[c,k]=[c<=k], B[c,j]=-20*[c>=j+1],
     (A^T B)[k,j] = -20*max(0, k-j), accumulated into the scores psum.
  3. exp on the Act engine, psum f32 -> SBUF bf16 (masked entries give
     exp(-20*d) ~ 0).
  4. attn@V flipped: stationary = e^T block [128k x 128q], moving =
     [V_h | ones] (65 cols) accumulated over k tiles -> psum [q, 64+1];
     col 64 is the softmax denominator.  Normalize during the PSUM->SBUF
     copy (per-partition reciprocal scale), giving ctx [seq, 256] bf16.
  5. ctx^T via 128x128 SBUF->SBUF DMA transposes (bf16), then the partial
     out-projection y = ctx @ W_o[slice] in bf16, psum f32 DMA'd straight
     to DRAM.
"""

import numpy as np
import ml_dtypes
from contextlib import ExitStack

import concourse.bass as bass
import concourse.bacc as bacc
import concourse.tile as tile
import concourse.mybir as mybir
from concourse.bass_utils import run_bass_kernel_spmd

F32 = mybir.dt.float32
BF16 = mybir.dt.bfloat16
FP8 = mybir.dt.float8e4
AF = mybir.ActivationFunctionType
ALU = mybir.AluOpType
DR = mybir.MatmulPerfMode.DoubleRow

NP_FP8 = ml_dtypes.float8_e4m3
NP_BF16 = ml_dtypes.bfloat16

B = 2
S = 2048
D = 1024
DC = 256          # head dims per core (4 heads x 64)
N_CORES = 8
NT = D // 128     # 8 input-dim tiles
ST = S // 128     # 16 sequence tiles
MSK = 20.0        # causal mask slope (per unit of k-q)


def _bank_slices(a, b):
    """Split columns [a, b) at 512-float PSUM bank boundaries."""
    out = []
    while a < b:
        nxt = min(b, (a // 512 + 1) * 512)
        out.append((a, nxt))
        a = nxt
    return out


def _build():
    nc = bacc.Bacc("TRN2", target_bir_lowering=False, debug=False,
                   num_devices=N_CORES)
    xt8 = nc.dram_tensor("xt8", [D, S], FP8, kind="ExternalInput").ap()
    xtr = nc.dram_tensor("xtr", [D, S], FP8, kind="ExternalInput").ap()
    w8 = {}
    wr = {}
    for nm in ("q", "k", "v"):
        w8[nm] = nc.dram_tensor(f"w{nm}8", [D, DC], FP8,
                                kind="ExternalInput").ap()
        wr[nm] = nc.dram_tensor(f"w{nm}r", [D, DC], FP8,
                                kind="ExternalInput").ap()
    wo = nc.dram_tensor("wo", [DC, D], BF16, kind="ExternalInput").ap()
    mka = nc.dram_tensor("mka", [128, 128], BF16, kind="ExternalInput").ap()
    mkb = nc.dram_tensor("mkb", [128, 128], BF16, kind="ExternalInput").ap()
    y = nc.dram_tensor("y", [S, D], F32, kind="ExternalOutput").ap()

    with tile.TileContext(nc) as tc, ExitStack() as stk:
        persist = stk.enter_context(tc.tile_pool(name="persist", bufs=1))
        x8_sb = persist.tile([128, NT, S], FP8)
        xr_sb = persist.tile([128, NT, S], FP8)
        w8_sb = {}
        wr_sb = {}
        for nm in ("q", "k", "v"):
            w8_sb[nm] = persist.tile([128, NT, DC], FP8, name=f"w8_{nm}")
            wr_sb[nm] = persist.tile([128, NT, DC], FP8, name=f"wr_{nm}")
        wo_sb = persist.tile([128, 2, D], BF16)
        mka_sb = persist.tile([128, 128], BF16)
        mkb_sb = persist.tile([128, 128], BF16)
        qt_sb = persist.tile([128, 2, S], BF16)   # Q^T: j-tile of dims x seq
        kt_sb = persist.tile([128, 2, S], BF16)
        v2_sb = persist.tile([128, ST, 4, 65], BF16)  # [k, ktile, head, V|1]
        ctx_sb = persist.tile([128, ST, DC], BF16)    # [q, qtile, dims]
        ctxT_sb = persist.tile([128, 2, S], BF16)     # [dims, dtile, seq]
        rcp_sb = persist.tile([128, 64], F32)         # 1/denom per (h,qt)

        # ---- input DMAs (SP queue) ----
        nc.sync.dma_start(out=mka_sb[:], in_=mka[:, :])
        nc.sync.dma_start(out=mkb_sb[:], in_=mkb[:, :])
        for nm in ("q", "k", "v"):
            for i in range(NT):
                nc.sync.dma_start(out=w8_sb[nm][:, i, :],
                                  in_=w8[nm][128 * i:128 * (i + 1), :])
                nc.sync.dma_start(out=wr_sb[nm][:, i, :],
                                  in_=wr[nm][128 * i:128 * (i + 1), :])
        for d in range(2):
            nc.sync.dma_start(out=wo_sb[:, d, :],
                              in_=wo[128 * d:128 * (d + 1), :])
        for i in range(NT):
            nc.sync.dma_start(out=x8_sb[:, i, :],
                              in_=xt8[128 * i:128 * (i + 1), :])
            nc.sync.dma_start(out=xr_sb[:, i, :],
                              in_=xtr[128 * i:128 * (i + 1), :])

        nc.vector.memset(v2_sb[:, :, :, 64:65], 1.0)

        # ---- projections ----
        with tc.tile_pool(name="pp", bufs=2, space="PSUM") as pp, \
             tc.tile_pool(name="pv", bufs=2, space="PSUM") as pv:

            def proj_qk(nm, j, half, dst):
                """Q^T/K^T tile: psum[128 dq, 1024 q] -> dst[:, j, half]."""
                ps = pp.tile([128, 1024], F32, tag="pp", name=f"p{nm}{j}{half}")
                terms = ((w8_sb[nm], x8_sb), (wr_sb[nm], x8_sb),
                         (w8_sb[nm], xr_sb))
                for t in range(NT // 2):
                    for ti, (wsb, xsb) in enumerate(terms):
                        for a, b in ((0, 512), (512, 1024)):
                            nc.tensor.matmul(
                                ps[:, a:b],
                                lhsT=wsb[:, 2 * t:2 * t + 2,
                                         128 * j:128 * (j + 1)],
                                rhs=xsb[:, 2 * t:2 * t + 2,
                                        1024 * half + a:1024 * half + b],
                                start=(t == 0 and ti == 0),
                                stop=(t == NT // 2 - 1 and ti == 2),
                                perf_mode=DR)
                scale = 1.0 / 512.0 if nm == "q" else 1.0 / 64.0
                nc.vector.tensor_scalar_mul(
                    dst[:, j, 1024 * half:1024 * (half + 1)], ps[:, :], scale)

            def proj_v(st):
                """V tile: psum[128 seq, 256 dv] -> v2_sb[:, st, :, 0:64]."""
                ps = pv.tile([128, 256], F32, tag="pv", name=f"pv{st}")
                terms = ((x8_sb, w8_sb["v"]), (x8_sb, wr_sb["v"]),
                         (xr_sb, w8_sb["v"]))
                n = 0
                for t in range(NT // 2):
                    for xsb, wsb in terms:
                        nc.tensor.matmul(
                            ps[:, 0:256],
                            lhsT=xsb[:, 2 * t:2 * t + 2,
                                     128 * st:128 * (st + 1)],
                            rhs=wsb[:, 2 * t:2 * t + 2, :],
                            start=(n == 0), stop=(n == 11),
                            perf_mode=DR)
                        n += 1
                nc.vector.tensor_scalar_mul(
                    v2_sb[:, st, :, 0:64],
                    ps[:, :].rearrange("p (h c) -> p h c", c=64),
                    1.0 / 64.0)

            # j=0 q/k first so attention can start early
            for half in range(2):
                proj_qk("q", 0, half, qt_sb)
            for half in range(2):
                proj_qk("k", 0, half, kt_sb)
            for st in range(4):
                proj_v(st)
            for half in range(2):
                proj_qk("q", 1, half, qt_sb)
            for half in range(2):
                proj_qk("k", 1, half, kt_sb)
            for st in range(4, ST):
                proj_v(st)

        # ---- attention + out-projection ----
        with tc.tile_pool(name="sp", bufs=2, space="PSUM") as sp, \
             tc.tile_pool(name="ap", bufs=2, space="PSUM") as apool, \
             tc.tile_pool(name="yp", bufs=2, space="PSUM") as yp, \
             tc.tile_pool(name="ep", bufs=2) as ep, \
             tc.tile_pool(name="yb", bufs=3) as yb:

            e_tiles = {}

            def scores_exp(h, qc):
                """S^T + mask into psum, exp -> e_sb[:, kt, :] (bf16)."""
                jh, hb = h // 2, 64 * (h % 2)
                e_sb = ep.tile([128, ST, 1024], BF16, tag="e", name=f"e{h}{qc}")
                e_tiles[(h, qc)] = e_sb
                for kt in range(8 * qc + 8):
                    off = max(0, 128 * kt - 1024 * qc)
                    s_ps = sp.tile([128, 1024], F32, tag="s",
                                   name=f"s{h}{qc}{kt}")
                    for a, b in _bank_slices(off, 1024):
                        nc.tensor.matmul(
                            s_ps[:, a:b],
                            lhsT=kt_sb[hb:hb + 64, jh,
                                       128 * kt:128 * (kt + 1)],
                            rhs=qt_sb[hb:hb + 64, jh,
                                      1024 * qc + a:1024 * qc + b],
                            start=True, stop=True)
                    if kt >= 8 * qc:
                        # diagonal 128-block: accumulate -MSK*max(0, k-q)
                        nc.tensor.matmul(
                            s_ps[:, off:off + 128],
                            lhsT=mka_sb[:, :], rhs=mkb_sb[:, :],
                            start=False, stop=True, skip_group_check=True)
                    nc.scalar.activation(e_sb[:, kt, off:1024],
                                         s_ps[:, off:1024], AF.Exp)

            def attn_v(h, qc):
                """ctx[q, 64]+denom per local q tile; normalize to ctx_sb."""
                e_sb = e_tiles.pop((h, qc))
                for qtl in range(8):
                    qt = 8 * qc + qtl
                    a_ps = apool.tile([128, 128], F32, tag="a",
                                      name=f"a{h}{qt}")
                    for kt in range(qt + 1):
                        nc.tensor.matmul(
                            a_ps[:, 0:65],
                            lhsT=e_sb[:, kt, 128 * qtl:128 * (qtl + 1)],
                            rhs=v2_sb[:, kt, h, :],
                            start=(kt == 0), stop=(kt == qt))
                    r = rcp_sb[:, 16 * h + qt:16 * h + qt + 1]
                    nc.vector.reciprocal(r, a_ps[:, 64:65])
                    nc.vector.tensor_scalar_mul(
                        ctx_sb[:, qt, 64 * h:64 * (h + 1)], a_ps[:, 0:64], r)

            def out_proj(qt):
                """ctx^T via DMA transpose, then y[qt] = ctx @ W_o slice."""
                for d in range(2):
                    nc.sync.dma_start_transpose(
                        out=ctxT_sb[:, d, 128 * qt:128 * (qt + 1)],
                        in_=ctx_sb[:, qt, 128 * d:128 * (d + 1)])
                for a, b in ((0, 512), (512, 1024)):
                    y_ps = yp.tile([128, 512], F32, tag="y", name=f"y{qt}{a}")
                    for d in range(2):
                        nc.tensor.matmul(
                            y_ps[:, :],
                            lhsT=ctxT_sb[:, d, 128 * qt:128 * (qt + 1)],
                            rhs=wo_sb[:, d, a:b],
                            start=(d == 0), stop=(d == 1))
                    y_sb = yb.tile([128, 512], F32, tag="ysb",
                                   name=f"ysb{qt}{a}")
                    nc.vector.tensor_copy(y_sb[:, :], y_ps[:, :])
                    nc.sync.dma_start(out=y[128 * qt:128 * (qt + 1), a:b],
                                      in_=y_sb[:, :])

            # software-pipelined emission: scores one block ahead of attn@V;
            # out-projection batches interleaved to keep engines fed.
            blocks = [(h, qc) for qc in range(2) for h in range(4)]
            scores_exp(*blocks[0])
            for i, blk in enumerate(blocks):
                if i + 1 < len(blocks):
                    scores_exp(*blocks[i + 1])
                attn_v(*blk)
                if blk == (1, 1):
                    for qt in range(8):
                        out_proj(qt)
            for qt in range(8, ST):
                out_proj(qt)

    nc.compile()
    return nc


_nc = None


def _quant_split(a):
    """fp8 value + unscaled fp8 residual."""
    hi = np.asarray(a, NP_FP8)
    lo = np.asarray(a - hi.astype(np.float32), NP_FP8)
    return hi, lo


def make_in_maps(x, W_q, W_k, W_v, W_o):
    mka = (np.arange(128)[:, None] <= np.arange(128)[None, :])
    mkb = -MSK * (np.arange(128)[:, None] >= np.arange(128)[None, :] + 1)
    mka = mka.astype(NP_BF16)
    mkb = mkb.astype(NP_BF16)
    in_maps = []
    for c in range(N_CORES):
        b, g = c // 4, c % 4
        sl = slice(DC * g, DC * (g + 1))
        x8, xr = _quant_split(np.ascontiguousarray(x[b].T))
        im = {"xt8": x8, "xtr": xr, "mka": mka, "mkb": mkb,
              "wo": np.ascontiguousarray(W_o[sl, :]).astype(NP_BF16)}
        for nm, W in (("q", W_q), ("k", W_k), ("v", W_v)):
            hi, lo = _quant_split(np.ascontiguousarray(W[:, sl]) * 64.0)
            im[f"w{nm}8"] = hi
            im[f"w{nm}r"] = lo
        in_maps.append(im)
    return in_maps


def kernel(x, W_q, W_k, W_v, W_o, b_o):
    global _nc
    x = np.asarray(x, dtype=np.float32)
    W_q = np.asarray(W_q, dtype=np.float32)
    W_k = np.asarray(W_k, dtype=np.float32)
    W_v = np.asarray(W_v, dtype=np.float32)
    W_o = np.asarray(W_o, dtype=np.float32)
    b_o = np.asarray(b_o, dtype=np.float32)

    if _nc is None:
        _nc = _build()

    in_maps = make_in_maps(x, W_q, W_k, W_v, W_o)
    res = run_bass_kernel_spmd(_nc, in_maps, list(range(N_CORES)))
    out = np.empty((B, S, D), dtype=np.float32)
    for b in range(B):
        acc = np.zeros((S, D), dtype=np.float64)
        for g in range(4):
            acc += res.results[4 * b + g]["y"]
        acc += b_o
        out[b] = acc.astype(np.float32)
    return out


# revision 14
# speedup vs baseline: 1.1980x; 1.1980x over previous
"""Multi-head causal attention (B=2, S=2048, D=1024, H=16, Dh=64) on 8 TRN2
NeuronCores.

Sharding: core c = 4*b + g handles batch b (2-way data parallel) and head
group g (4 heads = a 256-column slice of W_q/W_k/W_v, and the matching
256-row slice of W_o).  Each core returns a partial output [S, D]; the host
sums the 4 partials per batch and adds b_o.

On-core pipeline (per core):
  1. QKV projections as 3-term fp8 hi+lo DoubleRow matmuls:
     x@W ~= x8@W8 + x8@Wr + xr@W8 (residuals quantized unscaled), psum f32,
     descale during the PSUM->SBUF copy to bf16 (q also folds 1/sqrt(Dh)).
     Q^T/K^T land [dq, seq] (dims on partitions), V lands [seq, dv].
  2. Scores computed transposed per (head, 1024-query chunk, 128-key tile):
     S^T[k,q] = K^T.T Q^T in bf16.  The causal mask is folded in as one
     extra 128-wide matmul: with A[c,k]=[c<=k], B[c,j]=-20*[c>=j+1],
     (A^T B)[k,j] = -20*max(0, k-j), accumulated into the scores psum.
  3. exp on the Act engine, psum f32 -> SBUF bf16 (masked entries give
     exp(-20*d) ~ 0).
  4. attn@V flipped: stationary = e^T block [128k x 128q], moving =
     [V_h | ones] (65 cols) accumulated over k tiles -> psum [q, 64+1];
     col 64 is the softmax denominator.  Normalize during the PSUM->SBUF
     copy (per-partition reciprocal scale), giving ctx [seq, 256] bf16.
  5. ctx^T via 128x128 SBUF->SBUF DMA transposes (bf16), then the partial
     out-projection y = ctx @ W_o[slice] in bf16, psum f32 DMA'd straight
     to DRAM.
"""

import numpy as np
import ml_dtypes
from contextlib import ExitStack

import concourse.bass as bass
import concourse.bacc as bacc
import concourse.tile as tile
import concourse.mybir as mybir
from concourse.bass_utils import run_bass_kernel_spmd

F32 = mybir.dt.float32
BF16 = mybir.dt.bfloat16
FP8 = mybir.dt.float8e4
AF = mybir.ActivationFunctionType
ALU = mybir.AluOpType
DR = mybir.MatmulPerfMode.DoubleRow

NP_FP8 = ml_dtypes.float8_e4m3
NP_BF16 = ml_dtypes.bfloat16

B = 2
S = 2048
D = 1024
DC = 256          # head dims per core (4 heads x 64)
N_CORES = 8
NT = D // 128     # 8 input-dim tiles
ST = S // 128     # 16 sequence tiles
MSK = 20.0        # causal mask slope (per unit of k-q)


def _bank_slices(a, b):
    """Split columns [a, b) at 512-float PSUM bank boundaries."""
    out = []
    while a < b:
        nxt = min(b, (a // 512 + 1) * 512)
        out.append((a, nxt))
        a = nxt
    return out


def _build():
    nc = bacc.Bacc("TRN2", target_bir_lowering=False, debug=False,
                   num_devices=N_CORES)
    xt8 = nc.dram_tensor("xt8", [D, S], FP8, kind="ExternalInput").ap()
    xtr = nc.dram_tensor("xtr", [D, S], FP8, kind="ExternalInput").ap()
    w8 = {}
    wr = {}
    for nm in ("q", "k", "v"):
        w8[nm] = nc.dram_tensor(f"w{nm}8", [D, DC], FP8,
                                kind="ExternalInput").ap()
        wr[nm] = nc.dram_tensor(f"w{nm}r", [D, DC], FP8,
                                kind="ExternalInput").ap()
    wo = nc.dram_tensor("wo", [DC, D], BF16, kind="ExternalInput").ap()
    mka = nc.dram_tensor("mka", [128, 128], BF16, kind="ExternalInput").ap()
    mkb = nc.dram_tensor("mkb", [128, 128], BF16, kind="ExternalInput").ap()
    y = nc.dram_tensor("y", [S, D], F32, kind="ExternalOutput").ap()

    with tile.TileContext(nc) as tc, ExitStack() as stk:
        persist = stk.enter_context(tc.tile_pool(name="persist", bufs=1))
        x8_sb = persist.tile([128, NT, S], FP8)
        xr_sb = persist.tile([128, NT, S], FP8)
        w8_sb = {}
        wr_sb = {}
        for nm in ("q", "k", "v"):
            w8_sb[nm] = persist.tile([128, NT, DC], FP8, name=f"w8_{nm}")
            wr_sb[nm] = persist.tile([128, NT, DC], FP8, name=f"wr_{nm}")
        wo_sb = persist.tile([128, 2, D], BF16)
        mka_sb = persist.tile([128, 128], BF16)
        mkb_sb = persist.tile([128, 128], BF16)
        qt_sb = persist.tile([128, 2, S], BF16)   # Q^T: j-tile of dims x seq
        kt_sb = persist.tile([128, 2, S], BF16)
        v2_sb = persist.tile([128, ST, 4, 65], BF16)  # [k, ktile, head, V|1]
        ctx_sb = persist.tile([128, ST, DC], BF16)    # [q, qtile, dims]
        ctxT_sb = persist.tile([128, 2, S], BF16)     # [dims, dtile, seq]
        rcp_sb = persist.tile([128, 64], F32)         # 1/denom per (h,qt)

        # ---- input DMAs (SP queue), batched via 3D strided APs ----
        nc.sync.dma_start(out=mka_sb[:], in_=mka[:, :])
        nc.sync.dma_start(out=mkb_sb[:], in_=mkb[:, :])
        for nm in ("q", "k"):
            nc.sync.dma_start(out=w8_sb[nm][:, :, :],
                              in_=w8[nm].rearrange("(i p) c -> p i c", p=128))
            nc.sync.dma_start(out=wr_sb[nm][:, :, :],
                              in_=wr[nm].rearrange("(i p) c -> p i c", p=128))
        # x in 2-ktile chunks interleaved hi/lo so projections start early
        for t in range(NT // 2):
            nc.sync.dma_start(
                out=x8_sb[:, 2 * t:2 * t + 2, :],
                in_=xt8[256 * t:256 * (t + 1), :]
                .rearrange("(i p) s -> p i s", p=128))
            nc.sync.dma_start(
                out=xr_sb[:, 2 * t:2 * t + 2, :],
                in_=xtr[256 * t:256 * (t + 1), :]
                .rearrange("(i p) s -> p i s", p=128))
        nc.sync.dma_start(out=w8_sb["v"][:, :, :],
                          in_=w8["v"].rearrange("(i p) c -> p i c", p=128))
        nc.sync.dma_start(out=wr_sb["v"][:, :, :],
                          in_=wr["v"].rearrange("(i p) c -> p i c", p=128))
        nc.sync.dma_start(out=wo_sb[:, :, :],
                          in_=wo.rearrange("(d p) c -> p d c", p=128))

        for h in range(4):
            nc.vector.memset(v2_sb[:, :, h, 64:65], 1.0)

        # ---- whole pipeline shares one PSUM pool (tags: big/ap/sy) ----
        with tc.tile_pool(name="ps", bufs=2, space="PSUM") as pspool, \
             tc.tile_pool(name="ep", bufs=2) as ep, \
             tc.tile_pool(name="yb", bufs=3) as yb:

            def proj_qk(nm, j, half, dst):
                """Q^T/K^T tile: psum[128 dq, 1024 q] -> dst[:, j, half]."""
                ps = pspool.tile([128, 1024], F32, tag="big",
                 name=f"p{nm}{j}{half}")
                terms = ((w8_sb[nm], x8_sb), (wr_sb[nm], x8_sb),
                         (w8_sb[nm], xr_sb))
                for t in range(NT // 2):
                    for ti, (wsb, xsb) in enumerate(terms):
                        for a, b in ((0, 512), (512, 1024)):
                            nc.tensor.matmul(
                                ps[:, a:b],
                                lhsT=wsb[:, 2 * t:2 * t + 2,
                                         128 * j:128 * (j + 1)],
                                rhs=xsb[:, 2 * t:2 * t + 2,
                                        1024 * half + a:1024 * half + b],
                                start=(t == 0 and ti == 0),
                                stop=(t == NT // 2 - 1 and ti == 2),
                                perf_mode=DR)
                scale = 1.0 / 512.0 if nm == "q" else 1.0 / 64.0
                nc.vector.tensor_scalar_mul(
                    dst[:, j, 1024 * half:1024 * (half + 1)], ps[:, :], scale)

            def proj_v(st):
                """V tile: psum[128 seq, 256 dv] -> v2_sb[:, st, :, 0:64]."""
                ps = pspool.tile([128, 256], F32, tag="sy", name=f"pv{st}")
                terms = ((x8_sb, w8_sb["v"]), (x8_sb, wr_sb["v"]),
                         (xr_sb, w8_sb["v"]))
                n = 0
                for t in range(NT // 2):
                    for xsb, wsb in terms:
                        nc.tensor.matmul(
                            ps[:, 0:256],
                            lhsT=xsb[:, 2 * t:2 * t + 2,
                                     128 * st:128 * (st + 1)],
                            rhs=wsb[:, 2 * t:2 * t + 2, :],
                            start=(n == 0), stop=(n == 11),
                            perf_mode=DR)
                        n += 1
                nc.vector.tensor_scalar_mul(
                    v2_sb[:, st, :, 0:64],
                    ps[:, :].rearrange("p (h c) -> p h c", c=64),
                    1.0 / 64.0)


            # ---- attention + out-projection ----
            e_tiles = {}

            def scores_exp(h, qc):
                """S^T + mask into psum, exp -> e_sb[:, kt, :] (bf16)."""
                jh, hb = h // 2, 64 * (h % 2)
                e_sb = ep.tile([128, ST, 1024], BF16, tag="e", name=f"e{h}{qc}")
                e_tiles[(h, qc)] = e_sb
                for kt in range(8 * qc + 8):
                    off = max(0, 128 * kt - 1024 * qc)
                    s_ps = pspool.tile([128, 1024], F32, tag="big",
                                       name=f"s{h}{qc}{kt}")
                    for a, b in _bank_slices(off, 1024):
                        nc.tensor.matmul(
                            s_ps[:, a:b],
                            lhsT=kt_sb[hb:hb + 64, jh,
                                       128 * kt:128 * (kt + 1)],
                            rhs=qt_sb[hb:hb + 64, jh,
                                      1024 * qc + a:1024 * qc + b],
                            start=True, stop=True)
                    if kt >= 8 * qc:
                        # diagonal 128-block: accumulate -MSK*max(0, k-q)
                        nc.tensor.matmul(
                            s_ps[:, off:off + 128],
                            lhsT=mka_sb[:, :], rhs=mkb_sb[:, :],
                            start=False, stop=True, skip_group_check=True)
                    nc.scalar.activation(e_sb[:, kt, off:1024],
                                         s_ps[:, off:1024], AF.Exp)

            def attn_v(h, qc, per_qt=None):
                """ctx[q, 64]+denom per local q tile; normalize to ctx_sb."""
                e_sb = e_tiles.pop((h, qc))
                for qtl in range(8):
                    qt = 8 * qc + qtl
                    a_ps = pspool.tile([128, 128], F32, tag="ap",
                                       name=f"a{h}{qt}")
                    for kt in range(qt + 1):
                        nc.tensor.matmul(
                            a_ps[:, 0:65],
                            lhsT=e_sb[:, kt, 128 * qtl:128 * (qtl + 1)],
                            rhs=v2_sb[:, kt, h, :],
                            start=(kt == 0), stop=(kt == qt))
                    r = rcp_sb[:, 16 * h + qt:16 * h + qt + 1]
                    nc.vector.reciprocal(r, a_ps[:, 64:65])
                    nc.vector.tensor_scalar_mul(
                        ctx_sb[:, qt, 64 * h:64 * (h + 1)], a_ps[:, 0:64], r)
                    if per_qt is not None:
                        per_qt(qt)

            def out_proj(qt):
                """ctx^T via DMA transpose, then y[qt] = ctx @ W_o slice."""
                for d in range(2):
                    nc.sync.dma_start_transpose(
                        out=ctxT_sb[:, d, 128 * qt:128 * (qt + 1)],
                        in_=ctx_sb[:, qt, 128 * d:128 * (d + 1)])
                for a, b in ((0, 512), (512, 1024)):
                    y_ps = pspool.tile([128, 512], F32, tag="sy",
                   name=f"y{qt}{a}")
                    for d in range(2):
                        nc.tensor.matmul(
                            y_ps[:, :],
                            lhsT=ctxT_sb[:, d, 128 * qt:128 * (qt + 1)],
                            rhs=wo_sb[:, d, a:b],
                            start=(d == 0), stop=(d == 1))
                    y_sb = yb.tile([128, 512], F32, tag="ysb",
                                   name=f"ysb{qt}{a}")
                    nc.vector.tensor_copy(y_sb[:, :], y_ps[:, :])
                    nc.sync.dma_start(out=y[128 * qt:128 * (qt + 1), a:b],
                                      in_=y_sb[:, :])

            # Emission schedule: j0 projections, then j0-head attention
            # blocks whose Act-bound exp hides the j1 projections; scores
            # run one block ahead of attn@V; out-projection interleaved.
            for half in range(2):
                proj_qk("q", 0, half, qt_sb)
            for half in range(2):
                proj_qk("k", 0, half, kt_sb)
            for st in range(8):
                proj_v(st)

            scores_exp(0, 0)
            scores_exp(1, 0)
            attn_v(0, 0)
            scores_exp(0, 1)
            attn_v(1, 0)
            # j1 projections + remaining V hide under exp(0,1)/exp(1,1)
            for half in range(2):
                proj_qk("q", 1, half, qt_sb)
            scores_exp(1, 1)
            for half in range(2):
                proj_qk("k", 1, half, kt_sb)
            for st in range(8, ST):
                proj_v(st)
            attn_v(0, 1)
            scores_exp(2, 0)
            attn_v(1, 1)
            scores_exp(3, 0)
            attn_v(2, 0)
            scores_exp(2, 1)
            attn_v(3, 0)
            for qt in range(8):
                out_proj(qt)
            scores_exp(3, 1)
            attn_v(2, 1)
            attn_v(3, 1, per_qt=lambda qt: out_proj(qt))

    nc.compile()
    return nc


_nc = None


def _quant_split(a):
    """fp8 value + unscaled fp8 residual."""
    hi = np.asarray(a, NP_FP8)
    lo = np.asarray(a - hi.astype(np.float32), NP_FP8)
    return hi, lo


def make_in_maps(x, W_q, W_k, W_v, W_o):
    mka = (np.arange(128)[:, None] <= np.arange(128)[None, :])
    mkb = -MSK * (np.arange(128)[:, None] >= np.arange(128)[None, :] + 1)
    mka = mka.astype(NP_BF16)
    mkb = mkb.astype(NP_BF16)
    in_maps = []
    for c in range(N_CORES):
        b, g = c // 4, c % 4
        sl = slice(DC * g, DC * (g + 1))
        x8, xr = _quant_split(np.ascontiguousarray(x[b].T))
        im = {"xt8": x8, "xtr": xr, "mka": mka, "mkb": mkb,
              "wo": np.ascontiguousarray(W_o[sl, :]).astype(NP_BF16)}
        for nm, W in (("q", W_q), ("k", W_k), ("v", W_v)):
            hi, lo = _quant_split(np.ascontiguousarray(W[:, sl]) * 64.0)
            im[f"w{nm}8"] = hi
            im[f"w{nm}r"] = lo
        in_maps.append(im)
    return in_maps


def kernel(x, W_q, W_k, W_v, W_o, b_o):
    global _nc
    x = np.asarray(x, dtype=np.float32)
    W_q = np.asarray(W_q, dtype=np.float32)
    W_k = np.asarray(W_k, dtype=np.float32)
    W_v = np.asarray(W_v, dtype=np.float32)
    W_o = np.asarray(W_o, dtype=np.float32)
    b_o = np.asarray(b_o, dtype=np.float32)

    if _nc is None:
        _nc = _build()

    in_maps = make_in_maps(x, W_q, W_k, W_v, W_o)
    res = run_bass_kernel_spmd(_nc, in_maps, list(range(N_CORES)))
    out = np.empty((B, S, D), dtype=np.float32)
    for b in range(B):
        acc = np.zeros((S, D), dtype=np.float64)
        for g in range(4):
            acc += res.results[4 * b + g]["y"]
        acc += b_o
        out[b] = acc.astype(np.float32)
    return out


# revision 16
# speedup vs baseline: 1.3892x; 1.1596x over previous
"""Multi-head causal attention (B=2, S=2048, D=1024, H=16, Dh=64) on 8 TRN2
NeuronCores.

Sharding: core c = 4*b + g handles batch b (2-way data parallel) and head
group g (4 heads = a 256-column slice of W_q/W_k/W_v, and the matching
256-row slice of W_o).  Each core returns a partial output [S, D]; the host
sums the 4 partials per batch and adds b_o.

On-core pipeline (per core):
  1. QKV projections as 3-term fp8 hi+lo DoubleRow matmuls:
     x@W ~= x8@W8 + x8@Wr + xr@W8 (residuals quantized unscaled), psum f32,
     descale during the PSUM->SBUF copy to bf16 (q also folds 1/sqrt(Dh)).
     Q^T/K^T land [dq, seq] (dims on partitions), V lands [seq, dv].
  2. Scores computed transposed per (head, 1024-query chunk, 128-key tile):
     S^T[k,q] = K^T.T Q^T in bf16.  The causal mask is folded in as one
     extra 128-wide matmul: with A[c,k]=[c<=k], B[c,j]=-20*[c>=j+1],
     (A^T B)[k,j] = -20*max(0, k-j), accumulated into the scores psum.
  3. exp on the Act engine, psum f32 -> SBUF bf16 (masked entries give
     exp(-20*d) ~ 0).
  4. attn@V flipped: stationary = e^T block [128k x 128q], moving =
     [V_h | ones] (65 cols) accumulated over k tiles -> psum [q, 64+1];
     col 64 is the softmax denominator.  Normalize during the PSUM->SBUF
     copy (per-partition reciprocal scale), giving ctx [seq, 256] bf16.
  5. ctx^T via 128x128 SBUF->SBUF DMA transposes (bf16), then the partial
     out-projection y = ctx @ W_o[slice] in bf16, psum f32 DMA'd straight
     to DRAM.
"""

import numpy as np
import ml_dtypes
from contextlib import ExitStack

import concourse.bass as bass
import concourse.bacc as bacc
import concourse.tile as tile
import concourse.mybir as mybir
from concourse.bass_utils import run_bass_kernel_spmd

F32 = mybir.dt.float32
BF16 = mybir.dt.bfloat16
FP8 = mybir.dt.float8e4
AF = mybir.ActivationFunctionType
ALU = mybir.AluOpType
DR = mybir.MatmulPerfMode.DoubleRow

NP_FP8 = ml_dtypes.float8_e4m3
NP_BF16 = ml_dtypes.bfloat16

B = 2
S = 2048
D = 1024
DC = 256          # head dims per core (4 heads x 64)
N_CORES = 8
NT = D // 128     # 8 input-dim tiles
ST = S // 128     # 16 sequence tiles
MSK = 20.0        # causal mask slope (per unit of k-q)


def _bank_slices(a, b):
    """Split columns [a, b) at 512-float PSUM bank boundaries."""
    out = []
    while a < b:
        nxt = min(b, (a // 512 + 1) * 512)
        out.append((a, nxt))
        a = nxt
    return out


def _build():
    nc = bacc.Bacc("TRN2", target_bir_lowering=False, debug=False,
                   num_devices=N_CORES)
    xt8 = nc.dram_tensor("xt8", [D, S], FP8, kind="ExternalInput").ap()
    xtr = nc.dram_tensor("xtr", [D, S], FP8, kind="ExternalInput").ap()
    w8 = {}
    wr = {}
    for nm in ("q", "k", "v"):
        w8[nm] = nc.dram_tensor(f"w{nm}8", [D, DC], FP8,
                                kind="ExternalInput").ap()
        wr[nm] = nc.dram_tensor(f"w{nm}r", [D, DC], FP8,
                                kind="ExternalInput").ap()
    wo = nc.dram_tensor("wo", [DC, D], BF16, kind="ExternalInput").ap()
    mka = nc.dram_tensor("mka", [128, 128], BF16, kind="ExternalInput").ap()
    mkb = nc.dram_tensor("mkb", [128, 128], BF16, kind="ExternalInput").ap()
    y = nc.dram_tensor("y", [S, D], F32, kind="ExternalOutput").ap()

    with tile.TileContext(nc) as tc, ExitStack() as stk:
        persist = stk.enter_context(tc.tile_pool(name="persist", bufs=1))
        x8_sb = persist.tile([128, NT, S], FP8)
        xr_sb = persist.tile([128, NT, S], FP8)
        w8_sb = {}
        wr_sb = {}
        for nm in ("q", "k", "v"):
            w8_sb[nm] = persist.tile([128, NT, DC], FP8, name=f"w8_{nm}")
            wr_sb[nm] = persist.tile([128, NT, DC], FP8, name=f"wr_{nm}")
        wo_sb = persist.tile([128, 2, D], BF16)
        mka_sb = persist.tile([128, 128], BF16)
        mkb_sb = persist.tile([128, 128], BF16)
        qt_sb = persist.tile([128, 2, S], BF16)   # Q^T: j-tile of dims x seq
        kt_sb = persist.tile([128, 2, S], BF16)
        v2_sb = persist.tile([128, ST, 4, 65], BF16)  # [k, ktile, head, V|1]
        ctx_sb = persist.tile([128, ST, DC], BF16)    # [q, qtile, dims]
        ctxT_sb = persist.tile([128, 2, S], BF16)     # [dims, dtile, seq]
        rcp_sb = persist.tile([128, 64], F32)         # 1/denom per (h,qt)

        # ---- input DMAs (SP queue), batched via 3D strided APs ----
        nc.sync.dma_start(out=mka_sb[:], in_=mka[:, :])
        nc.sync.dma_start(out=mkb_sb[:], in_=mkb[:, :])
        nc.sync.dma_start(out=w8_sb["q"][:, :, :],
                          in_=w8["q"].rearrange("(i p) c -> p i c", p=128))
        nc.sync.dma_start(out=wr_sb["q"][:, :, :],
                          in_=wr["q"].rearrange("(i p) c -> p i c", p=128))
        # x in 2-ktile chunks interleaved hi/lo so projections start early
        for t in range(NT // 2):
            nc.sync.dma_start(
                out=x8_sb[:, 2 * t:2 * t + 2, :],
                in_=xt8[256 * t:256 * (t + 1), :]
                .rearrange("(i p) s -> p i s", p=128))
            nc.sync.dma_start(
                out=xr_sb[:, 2 * t:2 * t + 2, :],
                in_=xtr[256 * t:256 * (t + 1), :]
                .rearrange("(i p) s -> p i s", p=128))
        for nm in ("k", "v"):
            nc.sync.dma_start(out=w8_sb[nm][:, :, :],
                              in_=w8[nm].rearrange("(i p) c -> p i c", p=128))
            nc.sync.dma_start(out=wr_sb[nm][:, :, :],
                              in_=wr[nm].rearrange("(i p) c -> p i c", p=128))
        nc.sync.dma_start(out=wo_sb[:, :, :],
                          in_=wo.rearrange("(d p) c -> p d c", p=128))

        for h in range(4):
            nc.vector.memset(v2_sb[:, :, h, 64:65], 1.0)

        # ---- whole pipeline shares one PSUM pool (tags: big/ap/sy) ----
        with tc.tile_pool(name="ps", bufs=2, space="PSUM") as pspool, \
             tc.tile_pool(name="ep", bufs=2) as ep, \
             tc.tile_pool(name="yb", bufs=3) as yb:

            fillers = []

            def fill_one():
                if fillers:
                    fillers.pop(0)()

            def drain_fillers():
                while fillers:
                    fillers.pop(0)()

            def proj_qk_unit(nm, j, half, ab, dst):
                """One [128,512] slice of a Q^T/K^T tile: 12 DR matmuls."""
                a, b = ab
                ps = pspool.tile([128, 512], F32, tag="sy",
                                 name=f"p{nm}{j}{half}{a}")
                terms = ((w8_sb[nm], x8_sb), (wr_sb[nm], x8_sb),
                         (w8_sb[nm], xr_sb))
                for t in range(NT // 2):
                    for ti, (wsb, xsb) in enumerate(terms):
                        nc.tensor.matmul(
                            ps[:, :],
                            lhsT=wsb[:, 2 * t:2 * t + 2,
                                     128 * j:128 * (j + 1)],
                            rhs=xsb[:, 2 * t:2 * t + 2,
                                    1024 * half + a:1024 * half + b],
                            start=(t == 0 and ti == 0),
                            stop=(t == NT // 2 - 1 and ti == 2),
                            perf_mode=DR)
                scale = 1.0 / 512.0 if nm == "q" else 1.0 / 64.0
                nc.vector.tensor_scalar_mul(
                    dst[:, j, 1024 * half + a:1024 * half + b], ps[:, :],
                    scale)

            def proj_v(st):
                """V tile: psum[128 seq, 256 dv] -> v2_sb[:, st, :, 0:64]."""
                ps = pspool.tile([128, 256], F32, tag="sy", name=f"pv{st}")
                terms = ((x8_sb, w8_sb["v"]), (x8_sb, wr_sb["v"]),
                         (xr_sb, w8_sb["v"]))
                n = 0
                for t in range(NT // 2):
                    for xsb, wsb in terms:
                        nc.tensor.matmul(
                            ps[:, 0:256],
                            lhsT=xsb[:, 2 * t:2 * t + 2,
                                     128 * st:128 * (st + 1)],
                            rhs=wsb[:, 2 * t:2 * t + 2, :],
                            start=(n == 0), stop=(n == 11),
                            perf_mode=DR)
                        n += 1
                nc.vector.tensor_scalar_mul(
                    v2_sb[:, st, :, 0:64],
                    ps[:, :].rearrange("p (h c) -> p h c", c=64),
                    1.0 / 64.0)

            e_tiles = {}

            def scores_exp(h, qc, fill=False):
                """S^T + mask into psum, exp -> e_sb[:, kt, :] (bf16)."""
                jh, hb = h // 2, 64 * (h % 2)
                e_sb = ep.tile([128, ST, 1024], BF16, tag="e", name=f"e{h}{qc}")
                e_tiles[(h, qc)] = e_sb
                for kt in range(8 * qc + 8):
                    off = max(0, 128 * kt - 1024 * qc)
                    s_ps = pspool.tile([128, 1024], F32, tag="big",
                                       name=f"s{h}{qc}{kt}")
                    for a, b in _bank_slices(off, 1024):
                        nc.tensor.matmul(
                            s_ps[:, a:b],
                            lhsT=kt_sb[hb:hb + 64, jh,
                                       128 * kt:128 * (kt + 1)],
                            rhs=qt_sb[hb:hb + 64, jh,
                                      1024 * qc + a:1024 * qc + b],
                            start=True, stop=True)
                    if kt >= 8 * qc:
                        # diagonal 128-block: accumulate -MSK*max(0, k-q)
                        nc.tensor.matmul(
                            s_ps[:, off:off + 128],
                            lhsT=mka_sb[:, :], rhs=mkb_sb[:, :],
                            start=False, stop=True, skip_group_check=True)
                    nc.scalar.activation(e_sb[:, kt, off:1024],
                                         s_ps[:, off:1024], AF.Exp)
                    if fill:
                        fill_one()

            def attn_v(h, qc, per_qt=None, fill=False):
                """ctx[q, 64]+denom per local q tile; normalize to ctx_sb."""
                e_sb = e_tiles.pop((h, qc))
                for qtl in range(8):
                    qt = 8 * qc + qtl
                    a_ps = pspool.tile([128, 128], F32, tag="ap",
                                       name=f"a{h}{qt}")
                    for kt in range(qt + 1):
                        nc.tensor.matmul(
                            a_ps[:, 0:65],
                            lhsT=e_sb[:, kt, 128 * qtl:128 * (qtl + 1)],
                            rhs=v2_sb[:, kt, h, :],
                            start=(kt == 0), stop=(kt == qt))
                    r = rcp_sb[:, 16 * h + qt:16 * h + qt + 1]
                    nc.vector.reciprocal(r, a_ps[:, 64:65])
                    nc.vector.tensor_scalar_mul(
                        ctx_sb[:, qt, 64 * h:64 * (h + 1)], a_ps[:, 0:64], r)
                    if per_qt is not None:
                        per_qt(qt)
                    if fill:
                        fill_one()

            def out_proj(qt):
                """ctx^T via DMA transpose, then y[qt] = ctx @ W_o slice."""
                nc.sync.dma_start_transpose(
                    out=ctxT_sb[:, :, 128 * qt:128 * (qt + 1)],
                    in_=ctx_sb[:, qt, :])
                y_sb = yb.tile([128, 1024], F32, tag="ysb", name=f"ysb{qt}")
                for a, b in ((0, 512), (512, 1024)):
                    y_ps = pspool.tile([128, 512], F32, tag="sy",
                                       name=f"y{qt}{a}")
                    for d in range(2):
                        nc.tensor.matmul(
                            y_ps[:, :],
                            lhsT=ctxT_sb[:, d, 128 * qt:128 * (qt + 1)],
                            rhs=wo_sb[:, d, a:b],
                            start=(d == 0), stop=(d == 1))
                    nc.vector.tensor_copy(y_sb[:, a:b], y_ps[:, :])
                nc.sync.dma_start(out=y[128 * qt:128 * (qt + 1), :],
                                  in_=y_sb[:, :])

            # Emission schedule: j0 projections run directly (DMA-paced
            # startup); everything else enters as fine-grained fillers
            # consumed inside the Act-bound scores loops so the PE never
            # starves.  Scores run one block ahead of attn@V.
            halves = [(h, ab) for h in range(2) for ab in ((0, 512),
                                                           (512, 1024))]
            for half, ab in halves:
                proj_qk_unit("q", 0, half, ab, qt_sb)
            for half, ab in halves:
                proj_qk_unit("k", 0, half, ab, kt_sb)
            fillers += [lambda st=st: proj_v(st) for st in range(8)]

            scores_exp(0, 0, fill=True)
            scores_exp(1, 0, fill=True)
            drain_fillers()        # v[0..7] complete
            attn_v(0, 0)
            scores_exp(0, 1, fill=True)
            attn_v(1, 0)
            fillers += [lambda st=st: proj_v(st) for st in range(8, ST)]
            fillers += [lambda h=h, ab=ab: proj_qk_unit("q", 1, h, ab, qt_sb)
                        for h, ab in halves]
            fillers += [lambda h=h, ab=ab: proj_qk_unit("k", 1, h, ab, kt_sb)
                        for h, ab in halves]
            scores_exp(1, 1, fill=True)
            attn_v(0, 1, fill=True)
            drain_fillers()        # v[8..] + j1 q/k complete
            scores_exp(2, 0)
            attn_v(1, 1)
            scores_exp(3, 0)
            attn_v(2, 0)
            scores_exp(2, 1)
            attn_v(3, 0, per_qt=out_proj)      # out-proj qt 0..7
            scores_exp(3, 1)
            attn_v(2, 1)
            attn_v(3, 1, per_qt=out_proj)      # out-proj qt 8..15

    nc.compile()
    return nc


_nc = None


def _quant_split(a):
    """fp8 value + unscaled fp8 residual."""
    hi = np.asarray(a, NP_FP8)
    lo = np.asarray(a - hi.astype(np.float32), NP_FP8)
    return hi, lo


def make_in_maps(x, W_q, W_k, W_v, W_o):
    mka = (np.arange(128)[:, None] <= np.arange(128)[None, :])
    mkb = -MSK * (np.arange(128)[:, None] >= np.arange(128)[None, :] + 1)
    mka = mka.astype(NP_BF16)
    mkb = mkb.astype(NP_BF16)
    in_maps = []
    for c in range(N_CORES):
        b, g = c // 4, c % 4
        sl = slice(DC * g, DC * (g + 1))
        x8, xr = _quant_split(np.ascontiguousarray(x[b].T))
        im = {"xt8": x8, "xtr": xr, "mka": mka, "mkb": mkb,
              "wo": np.ascontiguousarray(W_o[sl, :]).astype(NP_BF16)}
        for nm, W in (("q", W_q), ("k", W_k), ("v", W_v)):
            hi, lo = _quant_split(np.ascontiguousarray(W[:, sl]) * 64.0)
            im[f"w{nm}8"] = hi
            im[f"w{nm}r"] = lo
        in_maps.append(im)
    return in_maps


def kernel(x, W_q, W_k, W_v, W_o, b_o):
    global _nc
    x = np.asarray(x, dtype=np.float32)
    W_q = np.asarray(W_q, dtype=np.float32)
    W_k = np.asarray(W_k, dtype=np.float32)
    W_v = np.asarray(W_v, dtype=np.float32)
    W_o = np.asarray(W_o, dtype=np.float32)
    b_o = np.asarray(b_o, dtype=np.float32)

    if _nc is None:
        _nc = _build()

    in_maps = make_in_maps(x, W_q, W_k, W_v, W_o)
    res = run_bass_kernel_spmd(_nc, in_maps, list(range(N_CORES)))
    out = np.empty((B, S, D), dtype=np.float32)
    for b in range(B):
        acc = np.zeros((S, D), dtype=np.float64)
        for g in range(4):
            acc += res.results[4 * b + g]["y"]
        acc += b_o
        out[b] = acc.astype(np.float32)
    return out
